# revision 7
# baseline (speedup 1.0000x reference)
"""DiffPOI on 8 trn2 cores: destination-sharded edge-parallel.
Random reads: dma_gather (int16, quarter tables). Reductions: one-hot
window matmuls over col-sorted padded streams (pad colrel=-1 -> no-op).
Dense W matmuls pushed to node shards via linearity of segment_sum.
Two launches; geo layer-1 table crosses cores via host (data movement).
Host does only integer index prep / sharding / layout."""
import numpy as np
import concourse.bacc as bacc
import concourse.mybir as mybir
import concourse.tile as tile
from concourse.bass_utils import run_bass_kernel_spmd

N_CORES = 8
P = 128
HID = 64
NPOI = 100000
NPAD = 100352
QN = 25088
SHN = 12544
NSESS = 200000
SESS_SH = 25000
SESS_PAD = 25088
WIN = 256
NIDX = 8192
F32 = mybir.dt.float32
BF16 = mybir.dt.bfloat16
I16 = mybir.dt.int16
AX = mybir.AxisListType.X
AF = mybir.ActivationFunctionType


def _wrap_data(a, dtype):
    return np.ascontiguousarray(np.asarray(a).reshape(-1, 128).T).astype(dtype)


def _wrap_idx(a):
    blk = np.ascontiguousarray(np.asarray(a).reshape(-1, 16).T).astype(np.int16)
    return np.tile(blk, (8, 1))


def _build_stream(keys_group, keys_win, arrays, pad_vals, seed_keys):
    """Uniform-across-cores padded streams.
    keys_group/keys_win: per-core lists of arrays. arrays: dict name -> list
    of per-core arrays. Every (group, window) segment is padded to the global
    max tile count so all cores share one program structure."""
    segs = {}
    per_core = []
    for c in range(N_CORES):
        g, w = keys_group[c], keys_win[c]
        order = np.lexsort((w, g))
        od = {k: v[c][order] for k, v in arrays.items()}
        og, ow = g[order], w[order]
        key = og.astype(np.int64) * 100000 + ow
        uk, starts, counts = np.unique(key, return_index=True, return_counts=True)
        per_core.append((od, uk, starts, counts))
        for u, cnt in zip(uk, counts):
            t = (int(cnt) + 127) // 128
            segs[int(u)] = max(segs.get(int(u), 0), t)
    for kk in seed_keys:
        segs[int(kk)] = max(segs.get(int(kk), 0), 1)
    seg_keys = sorted(segs)
    seg_tiles = [segs[k] for k in seg_keys]
    # per-core padded arrays
    out = {k: [] for k in arrays}
    for c in range(N_CORES):
        od, uk, starts, counts = per_core[c]
        ukl = {int(u): (int(s), int(cnt)) for u, s, cnt in zip(uk, starts, counts)}
        chunks = {k: [] for k in arrays}
        for kk, nt in zip(seg_keys, seg_tiles):
            want = nt * 128
            if kk in ukl:
                s, cnt = ukl[kk]
            else:
                s, cnt = 0, 0
            for name in arrays:
                seg = od[name][s:s + cnt]
                pad = np.full(want - cnt, pad_vals.get(name, 0), seg.dtype)
                chunks[name].append(np.concatenate([seg, pad]))
        for name in arrays:
            out[name].append(np.concatenate(chunks[name]))
    # program structure (same all cores)
    tile_grp = []
    tile_win = []
    for kk, nt in zip(seg_keys, seg_tiles):
        tile_grp += [kk // 100000] * nt
        tile_win += [kk % 100000] * nt
    # calls: whole segments, <= NIDX msgs, single group per call
    calls = []
    cur_g, cur_start, cur_tiles = tile_grp[0], 0, 0
    ti = 0
    for kk, nt in zip(seg_keys, seg_tiles):
        g = kk // 100000
        if g != cur_g or (cur_tiles + nt) * 128 > NIDX:
            if cur_tiles:
                calls.append((cur_g, cur_start * 128, cur_tiles * 128))
            cur_g, cur_start, cur_tiles = g, ti, 0
        cur_tiles += nt
        ti += nt
    if cur_tiles:
        calls.append((cur_g, cur_start * 128, cur_tiles * 128))
    L = sum(seg_tiles) * 128
    return out, tile_grp, tile_win, calls, L


def emit_reduce_call(nc, tc, sp, pp, lhs_tile, oh, k, tix0, tile_win, tile_grp,
                     flush):
    """Per-tile window matmuls with start/stop by segment; flush(ps, grp, win)
    at segment end. lhs_tile [P, k, M]."""
    ps = None
    for t in range(k):
        tix = tix0 + t
        w, g = tile_win[tix], tile_grp[tix]
        start_f = ps is None
        stop_f = (tix + 1 >= len(tile_win)) or tile_win[tix + 1] != w \
            or tile_grp[tix + 1] != g
        if start_f:
            ps = pp.tile([lhs_tile.shape[2], WIN], F32, tag="rps", name=f"rps{tix}")
        nc.tensor.matmul(out=ps[:], lhsT=lhs_tile[:, t, :], rhs=oh[:, t, :],
                         start=start_f, stop=stop_f)
        if stop_f:
            flush(ps, g, w)
            ps = None


def emit_geo(nc, tc, meta, aps, table_dram, aggT, iota):
    nc.vector.memset(aggT[:], 0.0)
    with tc.tile_pool(name="geo_p", bufs=1) as gp:
        with tc.tile_pool(name="geo_ps", bufs=2, space="PSUM") as pp:
            for (g, start, n) in meta["calls"]:
                k = n // P
                it = gp.tile([P, NIDX // 16], I16, tag="gi", name=f"gi{start}")
                nc.sync.dma_start(out=it[:, :n // 16],
                                  in_=aps["g_idx"][:, start // 16:(start + n) // 16])
                gt = gp.tile([P, NIDX // P, HID], F32, tag="gg", name=f"gg{start}")
                nc.gpsimd.dma_gather(
                    out_ap=gt[:, :k, :], in_ap=table_dram[g * QN:(g + 1) * QN, :],
                    idxs_ap=it[:, :n // 16], num_idxs=n, num_idxs_reg=n,
                    elem_size=HID, single_packet=False)
                dt = gp.tile([P, NIDX // P], F32, tag="gd", name=f"gd{start}")
                nc.sync.dma_start(out=dt[:, :k],
                                  in_=aps["g_dist"][:, start // P:(start + n) // P])
                nc.vector.tensor_mul(out=dt[:, :k], in0=dt[:, :k], in1=dt[:, :k])
                nc.scalar.activation(dt[:, :k], dt[:, :k], AF.Exp, scale=-1.0)
                gm = gp.tile([P, NIDX // P, HID], BF16, tag="gm", name=f"gm{start}")
                nc.vector.tensor_mul(
                    out=gm[:, :k, :], in0=gt[:, :k, :],
                    in1=dt[:, :k, None].to_broadcast([P, k, HID]))
                cr = gp.tile([P, NIDX // P], F32, tag="gc", name=f"gc{start}")
                nc.sync.dma_start(out=cr[:, :k],
                                  in_=aps["g_colrel"][:, start // P:(start + n) // P])
                oh = gp.tile([P, NIDX // P, WIN], BF16, tag="go", name=f"go{start}")
                nc.vector.tensor_tensor(
                    out=oh[:, :k, :], in0=cr[:, :k, None].to_broadcast([P, k, WIN]),
                    in1=iota[:, None, :].to_broadcast([P, k, WIN]),
                    op=mybir.AluOpType.is_equal)

                def gflush(ps, gg, w):
                    nc.vector.tensor_add(out=aggT[:, w * WIN:(w + 1) * WIN],
                                         in0=aggT[:, w * WIN:(w + 1) * WIN], in1=ps[:])

                emit_reduce_call(nc, tc, gp, pp, gm, oh, k, start // P,
                                 meta["tile_win"], meta["tile_grp"], gflush)


def emit_shard_mm(nc, tc, pool, aggT, WtT, dinv_sh, out_sh):
    with tc.tile_pool(name="hmm", bufs=2, space="PSUM") as pp:
        for c0 in range(0, SHN // P, 8):
            nb = min(8, SHN // P - c0)
            ps = pp.tile([P, 8 * HID], F32, tag="hps", name=f"hp{c0}")
            for t in range(nb):
                nc.tensor.matmul(out=ps[:, t * HID:(t + 1) * HID],
                                 lhsT=aggT[:, (c0 + t) * P:(c0 + t + 1) * P],
                                 rhs=WtT[:], start=True, stop=True)
            psv = ps[:].rearrange("p (b d) -> p b d", d=HID)
            nc.vector.tensor_mul(
                out=out_sh[:, c0:c0 + nb, :], in0=psv[:, :nb, :],
                in1=dinv_sh[:, c0:c0 + nb, None].to_broadcast([P, nb, HID]))
            sl = out_sh[:, c0:c0 + nb, :]
            tmp = pool.tile([P, 8, HID], F32, tag="ltmp", name=f"lt{c0}")
            nc.scalar.mul(out=tmp[:, :nb, :], in_=sl, mul=0.01)
            nc.vector.tensor_tensor(out=sl, in0=sl, in1=tmp[:, :nb, :],
                                    op=mybir.AluOpType.max)


def emit_table_scale(nc, tc, pool_unused, src_dram, dinv, table_dram):
    nb_tot = NPAD // P
    with tc.tile_pool(name="tscale", bufs=2) as pool:
        emit_table_scale_inner(nc, pool, src_dram, dinv, table_dram, nb_tot)


def emit_table_scale_inner(nc, pool, src_dram, dinv, table_dram, nb_tot):
    for b0 in range(0, nb_tot, 56):
        nb = min(56, nb_tot - b0)
        t = pool.tile([P, 56, HID], F32, tag="tstr", name=f"ts{b0}")
        src_v = src_dram.rearrange("(b p) d -> p b d", p=P)
        nc.sync.dma_start(out=t[:, :nb, :], in_=src_v[:, b0:b0 + nb, :])
        nc.vector.tensor_mul(
            out=t[:, :nb, :], in0=t[:, :nb, :],
            in1=dinv[:, b0:b0 + nb, None].to_broadcast([P, nb, HID]))
        dst_v = table_dram.rearrange("(b p) d -> p b d", p=P)
        nc.sync.dma_start(out=dst_v[:, b0:b0 + nb, :], in_=t[:, :nb, :])


def build_A(gmeta, smeta):
    nc = bacc.Bacc("TRN2", target_bir_lowering=False, debug=False, num_devices=N_CORES)
    GL, SL = gmeta["L"], smeta["L"]
    names = [
        ("poi", [NPAD, HID], F32), ("degf", [P, NPAD // P], F32),
        ("degsh", [P, SHN // P], F32), ("iota256", [P, WIN], F32),
        ("g_idx", [P, GL // 16], I16), ("g_dist", [P, GL // P], F32),
        ("g_colrel", [P, GL // P], F32), ("wt0", [HID, HID], F32),
        ("s_poij", [P, SL // 16], I16), ("s_xrel", [P, SL // 16], I16),
        ("s_dist", [P, SL // 16], I16), ("s_time", [P, SL // 16], I16),
        ("s_colrel", [P, SL // P], F32),
        ("x_sidx", [P, smeta["XL"] // 16], I16),
        ("alphas", [P, 2 * HID], F32),
        ("demb", [256, HID], F32), ("temb", [256, HID], F32),
    ]
    aps = {nm: nc.dram_tensor(nm, sh, dt, kind="ExternalInput").ap()
           for nm, sh, dt in names}
    l1sh_o = nc.dram_tensor("l1sh", [SHN, HID], F32, kind="ExternalOutput").ap()
    seqT_o = nc.dram_tensor("seqT", [HID + 1, SESS_PAD], F32, kind="ExternalOutput").ap()
    table1 = nc.dram_tensor("table1", [NPAD, HID], F32).ap()
    xsh = nc.dram_tensor("xsh", [smeta["XL"], HID], F32).ap()
    partials = nc.dram_tensor("partials", [8, HID + 1, SESS_PAD], F32).ap()

    with tile.TileContext(nc) as tc:
        with tc.tile_pool(name="base", bufs=1) as pool:
            iota = pool.tile([P, WIN], F32, tag="iota", name="iota_t")
            nc.sync.dma_start(out=iota[:], in_=aps["iota256"][:])
            dinv = pool.tile([P, NPAD // P], F32, tag="dinv", name="dinv_t")
            nc.sync.dma_start(out=dinv[:], in_=aps["degf"][:])
            nc.scalar.activation(dinv[:], dinv[:], AF.Sqrt)
            nc.vector.reciprocal(out=dinv[:], in_=dinv[:])
            emit_table_scale(nc, tc, pool, aps["poi"], dinv, table1)
            aggT = pool.tile([HID, SHN], F32, tag="aggT", name="aggT_t")
            emit_geo(nc, tc, gmeta, aps, table1, aggT, iota)
            wt0 = pool.tile([HID, HID], F32, tag="wt", name="wt0_t")
            nc.sync.dma_start(out=wt0[:], in_=aps["wt0"][:])
            dinv_sh = pool.tile([P, SHN // P], F32, tag="dsh", name="dsh_t")
            nc.sync.dma_start(out=dinv_sh[:], in_=aps["degsh"][:])
            nc.scalar.activation(dinv_sh[:], dinv_sh[:], AF.Sqrt)
            nc.vector.reciprocal(out=dinv_sh[:], in_=dinv_sh[:])
            l1sh = pool.tile([P, SHN // P, HID], F32, tag="l1", name="l1_t")
            emit_shard_mm(nc, tc, pool, aggT, wt0, dinv_sh, l1sh)
            l1v = l1sh_o.rearrange("(b p) d -> p b d", p=P)
            nc.sync.dma_start(out=l1v[:], in_=l1sh[:])
        with tc.tile_pool(name="seqp", bufs=1) as sp:
            iota2 = sp.tile([P, WIN], F32, tag="io2", name="io2_t")
            nc.sync.dma_start(out=iota2[:], in_=aps["iota256"][:])
            al = sp.tile([P, 2, HID], F32, tag="al", name="al_t")
            nc.sync.dma_start(out=al[:].rearrange("p a d -> p (a d)"), in_=aps["alphas"][:])
            for (g, start, n) in smeta["xcalls"]:
                k = n // P
                xi = sp.tile([P, NIDX // 16], I16, tag="xsi", name=f"xsi{start}")
                nc.sync.dma_start(out=xi[:, :n // 16],
                                  in_=aps["x_sidx"][:, start // 16:(start + n) // 16])
                xg = sp.tile([P, NIDX // P, HID], F32, tag="xsg", name=f"xsg{start}")
                nc.gpsimd.dma_gather(
                    out_ap=xg[:, :k, :], in_ap=aps["poi"][g * QN:(g + 1) * QN, :],
                    idxs_ap=xi[:, :n // 16], num_idxs=n, num_idxs_reg=n,
                    elem_size=HID, single_packet=False)
                xv = xsh.rearrange("(b p) d -> p b d", p=P)
                nc.sync.dma_start(out=xv[:, start // P:(start + n) // P, :],
                                  in_=xg[:, :k, :])
            stg = sp.tile([HID + 1, WIN], F32, tag="stg", name="stg_t")
            with tc.tile_pool(name="sps", bufs=2, space="PSUM") as pp:
                for (grp, start, n) in smeta["calls"]:
                    d, qj = grp // 4, grp % 4
                    k = n // P

                    def gath(tag, idx_ap, table_ap):
                        it = sp.tile([P, NIDX // 16], I16, tag=tag + "i", name=f"{tag}i{start}")
                        nc.sync.dma_start(out=it[:, :n // 16],
                                          in_=idx_ap[:, start // 16:(start + n) // 16])
                        g_ = sp.tile([P, NIDX // P, HID], F32, tag=tag, name=f"{tag}{start}")
                        nc.gpsimd.dma_gather(
                            out_ap=g_[:, :k, :], in_ap=table_ap, idxs_ap=it[:, :n // 16],
                            num_idxs=n, num_idxs_reg=n, elem_size=HID,
                            single_packet=False)
                        return g_

                    xj = gath("sxj", aps["s_poij"], aps["poi"][qj * QN:(qj + 1) * QN, :])
                    xi_ = gath("sxi", aps["s_xrel"], xsh[:])
                    el = gath("sel", aps["s_dist"], aps["demb"][:])
                    et = gath("set", aps["s_time"], aps["temb"][:])
                    sm = sp.tile([P, NIDX // P, HID], F32, tag="ssm", name=f"sm{start}")
                    nc.vector.tensor_mul(out=sm[:, :k, :], in0=xj[:, :k, :], in1=xi_[:, :k, :])
                    nc.vector.tensor_add(out=sm[:, :k, :], in0=sm[:, :k, :], in1=el[:, :k, :])
                    nc.vector.tensor_add(out=sm[:, :k, :], in0=sm[:, :k, :], in1=et[:, :k, :])
                    nc.vector.tensor_mul(
                        out=sm[:, :k, :], in0=sm[:, :k, :],
                        in1=al[:, d:d + 1, :].to_broadcast([P, k, HID]))
                    lg = sp.tile([P, NIDX // P], F32, tag="slg", name=f"lg{start}")
                    nc.vector.tensor_reduce(out=lg[:, :k], in_=sm[:, :k, :],
                                            axis=AX, op=mybir.AluOpType.add)
                    nc.scalar.activation(lg[:, :k], lg[:, :k], AF.Exp)
                    exm = sp.tile([P, NIDX // P, HID + 1], BF16, tag="sx", name=f"sx{start}")
                    nc.vector.tensor_mul(
                        out=exm[:, :k, :HID], in0=xj[:, :k, :],
                        in1=lg[:, :k, None].to_broadcast([P, k, HID]))
                    nc.vector.tensor_copy(out=exm[:, :k, HID:], in_=lg[:, :k, None])
                    cr = sp.tile([P, NIDX // P], F32, tag="scr", name=f"cr{start}")
                    nc.sync.dma_start(out=cr[:, :k],
                                      in_=aps["s_colrel"][:, start // P:(start + n) // P])
                    oh = sp.tile([P, NIDX // P, WIN], BF16, tag="soh", name=f"oh{start}")
                    nc.vector.tensor_tensor(
                        out=oh[:, :k, :], in0=cr[:, :k, None].to_broadcast([P, k, WIN]),
                        in1=iota2[:, None, :].to_broadcast([P, k, WIN]),
                        op=mybir.AluOpType.is_equal)

                    def sflush(ps, gg, w):
                        nc.vector.tensor_copy(out=stg[:], in_=ps[:])
                        nc.sync.dma_start(
                            out=partials[gg, :, w * WIN:(w + 1) * WIN], in_=stg[:])

                    emit_reduce_call(nc, tc, sp, pp, exm, oh, k, start // P,
                                     smeta["tile_win"], smeta["tile_grp"], sflush)
        with tc.tile_pool(name="comb", bufs=2) as cp:
            ones = cp.tile([1, HID], F32, tag="one", name="one_t")
            nc.vector.memset(ones[:], 1.0)
            with tc.tile_pool(name="cps", bufs=2, space="PSUM") as pp:
                CHK = 2048
                for w0 in range(0, SESS_PAD, CHK):
                    cw = min(CHK, SESS_PAD - w0)
                    acc = cp.tile([HID + 1, CHK], F32, tag="ca", name=f"ca{w0}")
                    nc.sync.dma_start(out=acc[:, :cw], in_=partials[0, :, w0:w0 + cw])
                    tmp = cp.tile([HID + 1, CHK], F32, tag="ctp", name=f"ct{w0}")
                    for g in range(1, 8):
                        nc.sync.dma_start(out=tmp[:, :cw], in_=partials[g, :, w0:w0 + cw])
                        nc.vector.tensor_add(out=acc[:, :cw], in0=acc[:, :cw], in1=tmp[:, :cw])
                    den = cp.tile([1, CHK], F32, tag="cd", name=f"cd{w0}")
                    nc.vector.tensor_scalar_add(out=den[:, :cw], in0=acc[HID:, :cw], scalar1=1e-16)
                    nc.vector.reciprocal(out=den[:, :cw], in_=den[:, :cw])
                    for s0 in range(0, cw, WIN):
                        ps = pp.tile([HID, WIN], F32, tag="cps", name=f"cp{w0}_{s0}")
                        nc.tensor.matmul(out=ps[:], lhsT=ones[:], rhs=den[:, s0:s0 + WIN],
                                         start=True, stop=True)
                        nc.vector.tensor_mul(out=acc[:HID, s0:s0 + WIN],
                                             in0=acc[:HID, s0:s0 + WIN], in1=ps[:])
                    nc.sync.dma_start(out=seqT_o[:, w0:w0 + cw], in_=acc[:, :cw])
    nc.compile()
    return nc


def build_B(gmeta):
    nc = bacc.Bacc("TRN2", target_bir_lowering=False, debug=False, num_devices=N_CORES)
    GL = gmeta["L"]
    names = [
        ("l1full", [NPAD, HID], F32), ("degf", [P, NPAD // P], F32),
        ("degsh", [P, SHN // P], F32), ("iota256", [P, WIN], F32),
        ("g_idx", [P, GL // 16], I16), ("g_dist", [P, GL // P], F32),
        ("g_colrel", [P, GL // P], F32), ("wt1", [HID, HID], F32),
        ("poish", [SHN, HID], F32), ("l1shr", [SHN, HID], F32),
    ]
    aps = {nm: nc.dram_tensor(nm, sh, dt, kind="ExternalInput").ap()
           for nm, sh, dt in names}
    geo_o = nc.dram_tensor("geosh", [SHN, HID], F32, kind="ExternalOutput").ap()
    table2 = nc.dram_tensor("table2", [NPAD, HID], F32).ap()
    with tile.TileContext(nc) as tc:
        with tc.tile_pool(name="base", bufs=1) as pool:
            iota = pool.tile([P, WIN], F32, tag="iota", name="iota_t")
            nc.sync.dma_start(out=iota[:], in_=aps["iota256"][:])
            dinv = pool.tile([P, NPAD // P], F32, tag="dinv", name="dinv_t")
            nc.sync.dma_start(out=dinv[:], in_=aps["degf"][:])
            nc.scalar.activation(dinv[:], dinv[:], AF.Sqrt)
            nc.vector.reciprocal(out=dinv[:], in_=dinv[:])
            emit_table_scale(nc, tc, pool, aps["l1full"], dinv, table2)
            aggT = pool.tile([HID, SHN], F32, tag="aggT", name="aggT_t")
            emit_geo(nc, tc, gmeta, aps, table2, aggT, iota)
            wt1 = pool.tile([HID, HID], F32, tag="wt", name="wt1_t")
            nc.sync.dma_start(out=wt1[:], in_=aps["wt1"][:])
            dinv_sh = pool.tile([P, SHN // P], F32, tag="dsh", name="dsh_t")
            nc.sync.dma_start(out=dinv_sh[:], in_=aps["degsh"][:])
            nc.scalar.activation(dinv_sh[:], dinv_sh[:], AF.Sqrt)
            nc.vector.reciprocal(out=dinv_sh[:], in_=dinv_sh[:])
            l2sh = pool.tile([P, SHN // P, HID], F32, tag="l2", name="l2_t")
            emit_shard_mm(nc, tc, pool, aggT, wt1, dinv_sh, l2sh)
            for b0 in range(0, SHN // P, 16):
                nb = min(16, SHN // P - b0)
                t1 = pool.tile([P, 16, HID], F32, tag="o1", name=f"o1{b0}")
                pv = aps["poish"].rearrange("(b p) d -> p b d", p=P)
                nc.sync.dma_start(out=t1[:, :nb, :], in_=pv[:, b0:b0 + nb, :])
                t2 = pool.tile([P, 16, HID], F32, tag="o2", name=f"o2{b0}")
                lv = aps["l1shr"].rearrange("(b p) d -> p b d", p=P)
                nc.sync.dma_start(out=t2[:, :nb, :], in_=lv[:, b0:b0 + nb, :])
                nc.vector.tensor_add(out=t1[:, :nb, :], in0=t1[:, :nb, :], in1=t2[:, :nb, :])
                nc.vector.tensor_add(out=t1[:, :nb, :], in0=t1[:, :nb, :],
                                     in1=l2sh[:, b0:b0 + nb, :])
                nc.scalar.mul(out=t1[:, :nb, :], in_=t1[:, :nb, :], mul=1.0 / 3.0)
                ov = geo_o.rearrange("(b p) d -> p b d", p=P)
                nc.sync.dma_start(out=ov[:, b0:b0 + nb, :], in_=t1[:, :nb, :])
    nc.compile()
    return nc


def kernel(poi_emb, distance_emb, temporal_emb, alpha_src, alpha_dst,
           W_geo, b_geo, geo_dist, geo_edge_index, sess_idx,
           seq_edge_index, edge_time, edge_dist):
    poi_emb = np.asarray(poi_emb, np.float32)
    geo_edge_index = np.asarray(geo_edge_index, np.int64)
    geo_dist = np.asarray(geo_dist, np.float32)
    sess_idx = np.asarray(sess_idx, np.int64)
    seq_edge_index = np.asarray(seq_edge_index, np.int64)
    edge_time = np.asarray(edge_time, np.int64)
    edge_dist_a = np.asarray(edge_dist, np.int64)

    poi_pad = np.zeros((NPAD, HID), np.float32)
    poi_pad[:NPOI] = poi_emb
    deg = np.bincount(geo_edge_index[1], minlength=NPOI) + 1
    deg_pad = np.ones(NPAD, np.float32)
    deg_pad[:NPOI] = deg
    degf = _wrap_data(deg_pad.reshape(NPAD // P, P).T.reshape(-1), np.float32)
    # careful: degf layout must match table row wrap (row = 128*b + p)
    degf = np.ascontiguousarray(deg_pad.reshape(NPAD // P, P).T).astype(np.float32)

    # ---- geo streams ----
    row = np.concatenate([geo_edge_index[0], np.arange(NPOI)])
    col = np.concatenate([geo_edge_index[1], np.arange(NPOI)])
    dist = np.concatenate([geo_dist, np.zeros(NPOI, np.float32)])
    shard = col // SHN
    kg, kw, arr_r, arr_d, arr_c = [], [], {"rowrel": [], "dist": [], "colrel": []}, None, None
    for c in range(N_CORES):
        m = shard == c
        r, cl, dd = row[m], col[m], dist[m]
        q = r // QN
        cll = cl - c * SHN
        w = cll // WIN
        kg.append(q)
        kw.append(w)
        arr_r["rowrel"].append((r - q * QN).astype(np.int64))
        arr_r["dist"].append(dd.astype(np.float32))
        arr_r["colrel"].append((cll - w * WIN).astype(np.float32))
    gseed = [g * 100000 + w for g in range(4) for w in range(SHN // WIN)]
    gout, g_tgrp, g_twin, g_calls, GL = _build_stream(
        kg, kw, arr_r, {"colrel": -1.0}, gseed)
    gmeta = {"L": GL, "calls": g_calls, "tile_grp": g_tgrp, "tile_win": g_twin}

    # ---- seq streams ----
    src, dst = seq_edge_index[0], seq_edge_index[1]
    j_all = np.concatenate([src, dst])
    i_all = np.concatenate([dst, src])
    d_all = np.concatenate([np.zeros(len(src), np.int64), np.ones(len(dst), np.int64)])
    e_all = np.concatenate([np.arange(len(src)), np.arange(len(src))])
    ish = i_all // SESS_SH
    ish[ish > 7] = 7
    skg, skw = [], []
    sarr = {"poijrel": [], "xrel": [], "edist": [], "etime": [], "colrel": []}
    xs_idx_list, sig_list = [], []
    for c in range(N_CORES):
        sess_lo = c * SESS_SH
        sl = sess_idx[sess_lo:sess_lo + SESS_SH]
        sig = np.argsort(sl // QN, kind="stable")
        inv = np.empty(SESS_SH, np.int64)
        inv[sig] = np.arange(SESS_SH)
        sig_list.append(sig)
        xs = np.zeros(SESS_PAD, np.int64)
        xs[:SESS_SH] = sl[sig] - (sl[sig] // QN) * QN
        xs_idx_list.append(xs)
        # x gather calls grouped by quarter (same structure all cores needed!)
        m = ish == c
        jm, im, dm, em = j_all[m], i_all[m], d_all[m], e_all[m]
        pj = sess_idx[jm]
        qj = pj // QN
        spos = inv[im - sess_lo]
        w = spos // WIN
        grp = dm * 4 + qj
        skg.append(grp)
        skw.append(w)
        sarr["poijrel"].append((pj - qj * QN).astype(np.int64))
        sarr["xrel"].append(spos.astype(np.int64))
        sarr["edist"].append(edge_dist_a[em].astype(np.int64))
        sarr["etime"].append(edge_time[em].astype(np.int64))
        sarr["colrel"].append((spos - w * WIN).astype(np.float32))
    sseed = [g * 100000 + w for g in range(8) for w in range(SESS_PAD // WIN)]
    sout, s_tgrp, s_twin, s_calls, SL = _build_stream(
        skg, skw, sarr, {"colrel": -1.0}, sseed)
    # x-shard gather calls: uniform via per-quarter max counts
    xqc = np.zeros((N_CORES, 4), np.int64)
    for c in range(N_CORES):
        q = (sess_idx[c * SESS_SH:(c + 1) * SESS_SH] // QN)
        for g in range(4):
            xqc[c, g] = (q == g).sum()
    xmax = [int(-(-xqc[:, g].max() // 128) * 128) for g in range(4)]
    XL = sum(xmax)
    xcalls = []
    pos = 0
    for g in range(4):
        off = 0
        while off < xmax[g]:
            n = min(NIDX, xmax[g] - off)
            xcalls.append((g, pos + off, n))
            off += n
        pos += xmax[g]
    smeta = {"L": SL, "calls": s_calls, "tile_grp": s_tgrp, "tile_win": s_twin,
             "xcalls": xcalls, "XL": XL}
    # per-core x_sidx padded to quarter-block structure
    xs_wrapped = []
    xpos_list = []
    for c in range(N_CORES):
        sl = sess_idx[c * SESS_SH:(c + 1) * SESS_SH]
        sig = sig_list[c]
        q_sorted = (sl[sig] // QN)
        buf = np.zeros(XL, np.int64)
        xpos = np.zeros(SESS_SH, np.int64)  # sigma-pos -> staged slot
        base = 0
        ptr = 0
        for g in range(4):
            cnt = int(xqc[c, g])
            buf[base:base + cnt] = (sl[sig] - q_sorted * QN)[ptr:ptr + cnt]
            xpos[ptr:ptr + cnt] = base + np.arange(cnt)
            ptr += cnt
            base += xmax[g]
        xs_wrapped.append(_wrap_idx(buf))
        xpos_list.append(xpos)
    # xrel must point at the STAGED slot (wrap layout [u%128, u//128])
    # staged row u lands at xsh row (u%128)*? no: we DMA tiles [128,k,64] to
    # xsh rows in (b p) order: row = 128*b + p = start + (u//128)*128?? tile
    # write xv[:, start/P + j, :] = gt[:, j, :] -> xsh row 128*(start/P+j)+p;
    # gathered msg u sits at [u%128, u//128] -> row = start + (u//128)*128 +
    # (u%128)?? NO: row index in (b p) wrap = 128*b + p with b = start//128 +
    # u//128, p = u%128 -> slot = start + 128*(u//128) + (u%128)... start is
    # mult of 128 so slot = start + u rearranged: slot_of_u = start +
    # (u//128)*128 + u%128 = start + u. Identity! Good: staged slot == buf pos.
    for c in range(N_CORES):
        spos = None  # xrel already = sigma position; remap to staged slot:
    # remap xrel arrays per core: staged_slot = xpos[sigma_pos]
    # (sout["xrel"] currently holds sigma positions)
    s_xrel_fixed = []
    for c in range(N_CORES):
        xr = sout["xrel"][c].astype(np.int64)
        xp = xpos_list[c]
        fixed = np.where(xr < SESS_SH, xp[np.clip(xr, 0, SESS_SH - 1)], 0)
        s_xrel_fixed.append(fixed)
    # NOTE: reduction windows are over sigma-pos space; output rows are in
    # sigma space?? No: colrel/w built from sigma positions (spos) -> seq
    # output row = sigma position. Host unpermutes via sig_list.

    iota256 = np.tile(np.arange(WIN, dtype=np.float32), (P, 1))
    alphas = np.tile(np.concatenate([np.asarray(alpha_src, np.float32),
                                     np.asarray(alpha_dst, np.float32)]), (P, 1))
    wt0 = np.ascontiguousarray(np.asarray(W_geo[0], np.float32).T)
    wt1 = np.ascontiguousarray(np.asarray(W_geo[1], np.float32).T)

    ncA = build_A(gmeta, smeta)
    in_maps = []
    for c in range(N_CORES):
        degsh = np.ascontiguousarray(
            deg_pad[c * SHN:(c + 1) * SHN].reshape(SHN // P, P).T).astype(np.float32)
        in_maps.append({
            "poi": poi_pad, "degf": degf, "degsh": degsh, "iota256": iota256,
            "g_idx": _wrap_idx(gout["rowrel"][c]),
            "g_dist": _wrap_data(gout["dist"][c], np.float32),
            "g_colrel": _wrap_data(gout["colrel"][c], np.float32),
            "wt0": wt0,
            "s_poij": _wrap_idx(sout["poijrel"][c]),
            "s_xrel": _wrap_idx(s_xrel_fixed[c]),
            "s_dist": _wrap_idx(sout["edist"][c]),
            "s_time": _wrap_idx(sout["etime"][c]),
            "s_colrel": _wrap_data(sout["colrel"][c], np.float32),
            "x_sidx": xs_wrapped[c],
            "alphas": alphas,
            "demb": np.asarray(distance_emb, np.float32),
            "temb": np.asarray(temporal_emb, np.float32),
        })
    resA = run_bass_kernel_spmd(ncA, in_maps, list(range(N_CORES)))

    l1_full = np.zeros((NPAD, HID), np.float32)
    for c in range(N_CORES):
        sh = resA.results[c]["l1sh"]
        l1_full[c * SHN:(c + 1) * SHN] = sh
    seq_out = np.zeros((NSESS, HID), np.float32)
    for c in range(N_CORES):
        sT = resA.results[c]["seqT"][:HID].T  # [SESS_PAD, 64] in sigma space
        sig = sig_list[c]
        out_rows = np.zeros((SESS_SH, HID), np.float32)
        out_rows[sig] = sT[:SESS_SH]
        seq_out[c * SESS_SH:(c + 1) * SESS_SH] = out_rows

    ncB = build_B(gmeta)
    in_mapsB = []
    for c in range(N_CORES):
        degsh = np.ascontiguousarray(
            deg_pad[c * SHN:(c + 1) * SHN].reshape(SHN // P, P).T).astype(np.float32)
        in_mapsB.append({
            "l1full": l1_full, "degf": degf, "degsh": degsh, "iota256": iota256,
            "g_idx": _wrap_idx(gout["rowrel"][c]),
            "g_dist": _wrap_data(gout["dist"][c], np.float32),
            "g_colrel": _wrap_data(gout["colrel"][c], np.float32),
            "wt1": wt1,
            "poish": poi_pad[c * SHN:(c + 1) * SHN],
            "l1shr": l1_full[c * SHN:(c + 1) * SHN],
        })
    resB = run_bass_kernel_spmd(ncB, in_mapsB, list(range(N_CORES)))
    geo_out = np.zeros((NPOI, HID), np.float32)
    for c in range(N_CORES):
        lo = c * SHN
        hi = min((c + 1) * SHN, NPOI)
        if lo < NPOI:
            geo_out[lo:hi] = resB.results[c]["geosh"][:hi - lo]
    return seq_out, geo_out


# revision 8
# speedup vs baseline: 6.6559x; 6.6559x over previous
"""DiffPOI on 8 trn2 cores: destination-sharded edge-parallel.
Random reads: dma_gather (int16, quarter tables). Reductions: one-hot
window matmuls over col-sorted padded streams (pad colrel=-1 -> no-op).
Dense W matmuls pushed to node shards via linearity of segment_sum.
Two launches; geo layer-1 table crosses cores via host (data movement).
Host does only integer index prep / sharding / layout."""
import numpy as np
import concourse.bacc as bacc
import concourse.mybir as mybir
import concourse.tile as tile
from concourse.bass_utils import run_bass_kernel_spmd

N_CORES = 8
P = 128
HID = 64
NPOI = 100000
NPAD = 100352
QN = 25088
SHN = 12544
NSESS = 200000
SESS_SH = 25000
SESS_PAD = 25088
WIN = 256
NIDX = 8192
F32 = mybir.dt.float32
BF16 = mybir.dt.bfloat16
I16 = mybir.dt.int16
AX = mybir.AxisListType.X
AF = mybir.ActivationFunctionType


def _wrap_data(a, dtype):
    return np.ascontiguousarray(np.asarray(a).reshape(-1, 128).T).astype(dtype)


def _wrap_idx(a):
    blk = np.ascontiguousarray(np.asarray(a).reshape(-1, 16).T).astype(np.int16)
    return np.tile(blk, (8, 1))


def _build_stream(keys_group, keys_win, arrays, pad_vals, seed_keys):
    """Uniform-across-cores padded streams.
    keys_group/keys_win: per-core lists of arrays. arrays: dict name -> list
    of per-core arrays. Every (group, window) segment is padded to the global
    max tile count so all cores share one program structure."""
    segs = {}
    per_core = []
    for c in range(N_CORES):
        g, w = keys_group[c], keys_win[c]
        order = np.lexsort((w, g))
        od = {k: v[c][order] for k, v in arrays.items()}
        og, ow = g[order], w[order]
        key = og.astype(np.int64) * 100000 + ow
        uk, starts, counts = np.unique(key, return_index=True, return_counts=True)
        per_core.append((od, uk, starts, counts))
        for u, cnt in zip(uk, counts):
            t = (int(cnt) + 127) // 128
            segs[int(u)] = max(segs.get(int(u), 0), t)
    for kk in seed_keys:
        segs[int(kk)] = max(segs.get(int(kk), 0), 1)
    seg_keys = sorted(segs)
    seg_tiles = [segs[k] for k in seg_keys]
    # per-core padded arrays
    out = {k: [] for k in arrays}
    for c in range(N_CORES):
        od, uk, starts, counts = per_core[c]
        ukl = {int(u): (int(s), int(cnt)) for u, s, cnt in zip(uk, starts, counts)}
        chunks = {k: [] for k in arrays}
        for kk, nt in zip(seg_keys, seg_tiles):
            want = nt * 128
            if kk in ukl:
                s, cnt = ukl[kk]
            else:
                s, cnt = 0, 0
            for name in arrays:
                seg = od[name][s:s + cnt]
                pad = np.full(want - cnt, pad_vals.get(name, 0), seg.dtype)
                chunks[name].append(np.concatenate([seg, pad]))
        for name in arrays:
            out[name].append(np.concatenate(chunks[name]))
    # program structure (same all cores)
    tile_grp = []
    tile_win = []
    for kk, nt in zip(seg_keys, seg_tiles):
        tile_grp += [kk // 100000] * nt
        tile_win += [kk % 100000] * nt
    # calls: whole segments, <= NIDX msgs, single group per call
    calls = []
    cur_g, cur_start, cur_tiles = tile_grp[0], 0, 0
    ti = 0
    for kk, nt in zip(seg_keys, seg_tiles):
        g = kk // 100000
        if g != cur_g or (cur_tiles + nt) * 128 > NIDX:
            if cur_tiles:
                calls.append((cur_g, cur_start * 128, cur_tiles * 128))
            cur_g, cur_start, cur_tiles = g, ti, 0
        cur_tiles += nt
        ti += nt
    if cur_tiles:
        calls.append((cur_g, cur_start * 128, cur_tiles * 128))
    L = sum(seg_tiles) * 128
    return out, tile_grp, tile_win, calls, L


def emit_reduce_call(nc, tc, sp, pp, lhs_tile, oh, k, tix0, tile_win, tile_grp,
                     flush):
    """Per-tile window matmuls with start/stop by segment; flush(ps, grp, win)
    at segment end. lhs_tile [P, k, M]."""
    ps = None
    for t in range(k):
        tix = tix0 + t
        w, g = tile_win[tix], tile_grp[tix]
        start_f = ps is None
        stop_f = (tix + 1 >= len(tile_win)) or tile_win[tix + 1] != w \
            or tile_grp[tix + 1] != g
        if start_f:
            ps = pp.tile([lhs_tile.shape[2], WIN], F32, tag="rps", name=f"rps{tix}")
        nc.tensor.matmul(out=ps[:], lhsT=lhs_tile[:, t, :], rhs=oh[:, t, :],
                         start=start_f, stop=stop_f)
        if stop_f:
            flush(ps, g, w)
            ps = None


def emit_geo(nc, tc, meta, aps, table_dram, aggT, iota):
    nc.vector.memset(aggT[:], 0.0)
    with tc.tile_pool(name="geo_p", bufs=1) as gp:
        with tc.tile_pool(name="geo_ps", bufs=2, space="PSUM") as pp:
            for (g, start, n) in meta["calls"]:
                k = n // P
                it = gp.tile([P, NIDX // 16], I16, tag="gi", name=f"gi{start}")
                nc.sync.dma_start(out=it[:, :n // 16],
                                  in_=aps["g_idx"][:, start // 16:(start + n) // 16])
                gt = gp.tile([P, NIDX // P, HID], F32, tag="gg", name=f"gg{start}")
                nc.gpsimd.dma_gather(
                    out_ap=gt[:, :k, :], in_ap=table_dram[g * QN:(g + 1) * QN, :],
                    idxs_ap=it[:, :n // 16], num_idxs=n, num_idxs_reg=n,
                    elem_size=HID, single_packet=False)
                dt = gp.tile([P, NIDX // P], F32, tag="gd", name=f"gd{start}")
                nc.sync.dma_start(out=dt[:, :k],
                                  in_=aps["g_dist"][:, start // P:(start + n) // P])
                nc.vector.tensor_mul(out=dt[:, :k], in0=dt[:, :k], in1=dt[:, :k])
                nc.scalar.activation(dt[:, :k], dt[:, :k], AF.Exp, scale=-1.0)
                gm = gp.tile([P, NIDX // P, HID], BF16, tag="gm", name=f"gm{start}")
                nc.vector.tensor_mul(
                    out=gm[:, :k, :], in0=gt[:, :k, :],
                    in1=dt[:, :k, None].to_broadcast([P, k, HID]))
                cr = gp.tile([P, NIDX // P], F32, tag="gc", name=f"gc{start}")
                nc.sync.dma_start(out=cr[:, :k],
                                  in_=aps["g_colrel"][:, start // P:(start + n) // P])
                oh = gp.tile([P, NIDX // P, WIN], BF16, tag="go", name=f"go{start}")
                nc.vector.tensor_tensor(
                    out=oh[:, :k, :], in0=cr[:, :k, None].to_broadcast([P, k, WIN]),
                    in1=iota[:, None, :].to_broadcast([P, k, WIN]),
                    op=mybir.AluOpType.is_equal)

                def gflush(ps, gg, w):
                    nc.vector.tensor_add(out=aggT[:, w * WIN:(w + 1) * WIN],
                                         in0=aggT[:, w * WIN:(w + 1) * WIN], in1=ps[:])

                emit_reduce_call(nc, tc, gp, pp, gm, oh, k, start // P,
                                 meta["tile_win"], meta["tile_grp"], gflush)


def emit_shard_mm(nc, tc, pool, aggT, WtT, dinv_sh, out_sh):
    with tc.tile_pool(name="hmm", bufs=2, space="PSUM") as pp:
        for c0 in range(0, SHN // P, 8):
            nb = min(8, SHN // P - c0)
            ps = pp.tile([P, 8 * HID], F32, tag="hps", name=f"hp{c0}")
            for t in range(nb):
                nc.tensor.matmul(out=ps[:, t * HID:(t + 1) * HID],
                                 lhsT=aggT[:, (c0 + t) * P:(c0 + t + 1) * P],
                                 rhs=WtT[:], start=True, stop=True)
            psv = ps[:].rearrange("p (b d) -> p b d", d=HID)
            nc.vector.tensor_mul(
                out=out_sh[:, c0:c0 + nb, :], in0=psv[:, :nb, :],
                in1=dinv_sh[:, c0:c0 + nb, None].to_broadcast([P, nb, HID]))
            sl = out_sh[:, c0:c0 + nb, :]
            tmp = pool.tile([P, 8, HID], F32, tag="ltmp", name=f"lt{c0}")
            nc.scalar.mul(out=tmp[:, :nb, :], in_=sl, mul=0.01)
            nc.vector.tensor_tensor(out=sl, in0=sl, in1=tmp[:, :nb, :],
                                    op=mybir.AluOpType.max)


def emit_table_scale(nc, tc, pool_unused, src_dram, dinv, table_dram):
    nb_tot = NPAD // P
    with tc.tile_pool(name="tscale", bufs=2) as pool:
        emit_table_scale_inner(nc, pool, src_dram, dinv, table_dram, nb_tot)


def emit_table_scale_inner(nc, pool, src_dram, dinv, table_dram, nb_tot):
    for b0 in range(0, nb_tot, 56):
        nb = min(56, nb_tot - b0)
        t = pool.tile([P, 56, HID], F32, tag="tstr", name=f"ts{b0}")
        src_v = src_dram.rearrange("(b p) d -> p b d", p=P)
        nc.sync.dma_start(out=t[:, :nb, :], in_=src_v[:, b0:b0 + nb, :])
        nc.vector.tensor_mul(
            out=t[:, :nb, :], in0=t[:, :nb, :],
            in1=dinv[:, b0:b0 + nb, None].to_broadcast([P, nb, HID]))
        dst_v = table_dram.rearrange("(b p) d -> p b d", p=P)
        nc.sync.dma_start(out=dst_v[:, b0:b0 + nb, :], in_=t[:, :nb, :])


def build_A(gmeta, smeta):
    nc = bacc.Bacc("TRN2", target_bir_lowering=False, debug=False, num_devices=N_CORES)
    GL, SL = gmeta["L"], smeta["L"]
    names = [
        ("poi", [NPAD, HID], F32), ("degf", [P, NPAD // P], F32),
        ("degsh", [P, SHN // P], F32), ("iota256", [P, WIN], F32),
        ("g_idx", [P, GL // 16], I16), ("g_dist", [P, GL // P], F32),
        ("g_colrel", [P, GL // P], F32), ("wt0", [HID, HID], F32),
        ("s_poij", [P, SL // 16], I16), ("s_xrel", [P, SL // 16], I16),
        ("s_dist", [P, SL // 16], I16), ("s_time", [P, SL // 16], I16),
        ("s_colrel", [P, SL // P], F32),
        ("x_sidx", [P, smeta["XL"] // 16], I16),
        ("alphas", [P, 2 * HID], F32),
        ("demb", [256, HID], F32), ("temb", [256, HID], F32),
    ]
    aps = {nm: nc.dram_tensor(nm, sh, dt, kind="ExternalInput").ap()
           for nm, sh, dt in names}
    l1sh_o = nc.dram_tensor("l1sh", [SHN, HID], F32, kind="ExternalOutput").ap()
    seqT_o = nc.dram_tensor("seqT", [HID + 1, SESS_PAD], F32, kind="ExternalOutput").ap()
    table1 = nc.dram_tensor("table1", [NPAD, HID], F32).ap()
    xsh = nc.dram_tensor("xsh", [smeta["XL"], HID], F32).ap()
    partials = nc.dram_tensor("partials", [8, HID + 1, SESS_PAD], F32).ap()

    with tile.TileContext(nc) as tc:
        with tc.tile_pool(name="base", bufs=1) as pool:
            iota = pool.tile([P, WIN], F32, tag="iota", name="iota_t")
            nc.sync.dma_start(out=iota[:], in_=aps["iota256"][:])
            dinv = pool.tile([P, NPAD // P], F32, tag="dinv", name="dinv_t")
            nc.sync.dma_start(out=dinv[:], in_=aps["degf"][:])
            nc.scalar.activation(dinv[:], dinv[:], AF.Sqrt)
            nc.vector.reciprocal(out=dinv[:], in_=dinv[:])
            emit_table_scale(nc, tc, pool, aps["poi"], dinv, table1)
            aggT = pool.tile([HID, SHN], F32, tag="aggT", name="aggT_t")
            emit_geo(nc, tc, gmeta, aps, table1, aggT, iota)
            wt0 = pool.tile([HID, HID], F32, tag="wt", name="wt0_t")
            nc.sync.dma_start(out=wt0[:], in_=aps["wt0"][:])
            dinv_sh = pool.tile([P, SHN // P], F32, tag="dsh", name="dsh_t")
            nc.sync.dma_start(out=dinv_sh[:], in_=aps["degsh"][:])
            nc.scalar.activation(dinv_sh[:], dinv_sh[:], AF.Sqrt)
            nc.vector.reciprocal(out=dinv_sh[:], in_=dinv_sh[:])
            l1sh = pool.tile([P, SHN // P, HID], F32, tag="l1", name="l1_t")
            emit_shard_mm(nc, tc, pool, aggT, wt0, dinv_sh, l1sh)
            l1v = l1sh_o.rearrange("(b p) d -> p b d", p=P)
            nc.sync.dma_start(out=l1v[:], in_=l1sh[:])
        with tc.tile_pool(name="seqp", bufs=1) as sp:
            iota2 = sp.tile([P, WIN], F32, tag="io2", name="io2_t")
            nc.sync.dma_start(out=iota2[:], in_=aps["iota256"][:])
            al = sp.tile([P, 2, HID], F32, tag="al", name="al_t")
            nc.sync.dma_start(out=al[:].rearrange("p a d -> p (a d)"), in_=aps["alphas"][:])
            for (g, start, n) in smeta["xcalls"]:
                k = n // P
                xi = sp.tile([P, NIDX // 16], I16, tag="xsi", name=f"xsi{start}")
                nc.sync.dma_start(out=xi[:, :n // 16],
                                  in_=aps["x_sidx"][:, start // 16:(start + n) // 16])
                xg = sp.tile([P, NIDX // P, HID], F32, tag="xsg", name=f"xsg{start}")
                nc.gpsimd.dma_gather(
                    out_ap=xg[:, :k, :], in_ap=aps["poi"][g * QN:(g + 1) * QN, :],
                    idxs_ap=xi[:, :n // 16], num_idxs=n, num_idxs_reg=n,
                    elem_size=HID, single_packet=False)
                xv = xsh.rearrange("(b p) d -> p b d", p=P)
                nc.sync.dma_start(out=xv[:, start // P:(start + n) // P, :],
                                  in_=xg[:, :k, :])
            stg = sp.tile([HID + 1, WIN], F32, tag="stg", name="stg_t")
            with tc.tile_pool(name="sps", bufs=2, space="PSUM") as pp:
                for (grp, start, n) in smeta["calls"]:
                    d, qj = grp // 4, grp % 4
                    k = n // P

                    def gath(tag, idx_ap, table_ap):
                        it = sp.tile([P, NIDX // 16], I16, tag=tag + "i", name=f"{tag}i{start}")
                        nc.sync.dma_start(out=it[:, :n // 16],
                                          in_=idx_ap[:, start // 16:(start + n) // 16])
                        g_ = sp.tile([P, NIDX // P, HID], F32, tag=tag, name=f"{tag}{start}")
                        nc.gpsimd.dma_gather(
                            out_ap=g_[:, :k, :], in_ap=table_ap, idxs_ap=it[:, :n // 16],
                            num_idxs=n, num_idxs_reg=n, elem_size=HID,
                            single_packet=False)
                        return g_

                    xj = gath("sxj", aps["s_poij"], aps["poi"][qj * QN:(qj + 1) * QN, :])
                    xi_ = gath("sxi", aps["s_xrel"], xsh[:])
                    el = gath("sel", aps["s_dist"], aps["demb"][:])
                    et = gath("set", aps["s_time"], aps["temb"][:])
                    sm = sp.tile([P, NIDX // P, HID], F32, tag="ssm", name=f"sm{start}")
                    nc.vector.tensor_mul(out=sm[:, :k, :], in0=xj[:, :k, :], in1=xi_[:, :k, :])
                    nc.vector.tensor_add(out=sm[:, :k, :], in0=sm[:, :k, :], in1=el[:, :k, :])
                    nc.vector.tensor_add(out=sm[:, :k, :], in0=sm[:, :k, :], in1=et[:, :k, :])
                    nc.vector.tensor_mul(
                        out=sm[:, :k, :], in0=sm[:, :k, :],
                        in1=al[:, d:d + 1, :].to_broadcast([P, k, HID]))
                    lg = sp.tile([P, NIDX // P], F32, tag="slg", name=f"lg{start}")
                    nc.vector.tensor_reduce(out=lg[:, :k], in_=sm[:, :k, :],
                                            axis=AX, op=mybir.AluOpType.add)
                    nc.scalar.activation(lg[:, :k], lg[:, :k], AF.Exp)
                    exm = sp.tile([P, NIDX // P, HID + 1], BF16, tag="sx", name=f"sx{start}")
                    nc.vector.tensor_mul(
                        out=exm[:, :k, :HID], in0=xj[:, :k, :],
                        in1=lg[:, :k, None].to_broadcast([P, k, HID]))
                    nc.vector.tensor_copy(out=exm[:, :k, HID:], in_=lg[:, :k, None])
                    cr = sp.tile([P, NIDX // P], F32, tag="scr", name=f"cr{start}")
                    nc.sync.dma_start(out=cr[:, :k],
                                      in_=aps["s_colrel"][:, start // P:(start + n) // P])
                    oh = sp.tile([P, NIDX // P, WIN], BF16, tag="soh", name=f"oh{start}")
                    nc.vector.tensor_tensor(
                        out=oh[:, :k, :], in0=cr[:, :k, None].to_broadcast([P, k, WIN]),
                        in1=iota2[:, None, :].to_broadcast([P, k, WIN]),
                        op=mybir.AluOpType.is_equal)

                    def sflush(ps, gg, w):
                        nc.vector.tensor_copy(out=stg[:], in_=ps[:])
                        nc.sync.dma_start(
                            out=partials[gg, :, w * WIN:(w + 1) * WIN], in_=stg[:])

                    emit_reduce_call(nc, tc, sp, pp, exm, oh, k, start // P,
                                     smeta["tile_win"], smeta["tile_grp"], sflush)
        with tc.tile_pool(name="comb", bufs=2) as cp:
            ones = cp.tile([1, HID], F32, tag="one", name="one_t")
            nc.vector.memset(ones[:], 1.0)
            with tc.tile_pool(name="cps", bufs=2, space="PSUM") as pp:
                CHK = 2048
                for w0 in range(0, SESS_PAD, CHK):
                    cw = min(CHK, SESS_PAD - w0)
                    acc = cp.tile([HID + 1, CHK], F32, tag="ca", name=f"ca{w0}")
                    nc.sync.dma_start(out=acc[:, :cw], in_=partials[0, :, w0:w0 + cw])
                    tmp = cp.tile([HID + 1, CHK], F32, tag="ctp", name=f"ct{w0}")
                    for g in range(1, 8):
                        nc.sync.dma_start(out=tmp[:, :cw], in_=partials[g, :, w0:w0 + cw])
                        nc.vector.tensor_add(out=acc[:, :cw], in0=acc[:, :cw], in1=tmp[:, :cw])
                    den = cp.tile([1, CHK], F32, tag="cd", name=f"cd{w0}")
                    nc.vector.tensor_scalar_add(out=den[:, :cw], in0=acc[HID:, :cw], scalar1=1e-16)
                    nc.vector.reciprocal(out=den[:, :cw], in_=den[:, :cw])
                    for s0 in range(0, cw, WIN):
                        ps = pp.tile([HID, WIN], F32, tag="cps", name=f"cp{w0}_{s0}")
                        nc.tensor.matmul(out=ps[:], lhsT=ones[:], rhs=den[:, s0:s0 + WIN],
                                         start=True, stop=True)
                        nc.vector.tensor_mul(out=acc[:HID, s0:s0 + WIN],
                                             in0=acc[:HID, s0:s0 + WIN], in1=ps[:])
                    nc.sync.dma_start(out=seqT_o[:, w0:w0 + cw], in_=acc[:, :cw])
    nc.compile()
    return nc


def build_B(gmeta):
    nc = bacc.Bacc("TRN2", target_bir_lowering=False, debug=False, num_devices=N_CORES)
    GL = gmeta["L"]
    names = [
        ("l1full", [NPAD, HID], F32), ("degf", [P, NPAD // P], F32),
        ("degsh", [P, SHN // P], F32), ("iota256", [P, WIN], F32),
        ("g_idx", [P, GL // 16], I16), ("g_dist", [P, GL // P], F32),
        ("g_colrel", [P, GL // P], F32), ("wt1", [HID, HID], F32),
        ("poish", [SHN, HID], F32), ("l1shr", [SHN, HID], F32),
    ]
    aps = {nm: nc.dram_tensor(nm, sh, dt, kind="ExternalInput").ap()
           for nm, sh, dt in names}
    geo_o = nc.dram_tensor("geosh", [SHN, HID], F32, kind="ExternalOutput").ap()
    table2 = nc.dram_tensor("table2", [NPAD, HID], F32).ap()
    with tile.TileContext(nc) as tc:
        with tc.tile_pool(name="base", bufs=1) as pool:
            iota = pool.tile([P, WIN], F32, tag="iota", name="iota_t")
            nc.sync.dma_start(out=iota[:], in_=aps["iota256"][:])
            dinv = pool.tile([P, NPAD // P], F32, tag="dinv", name="dinv_t")
            nc.sync.dma_start(out=dinv[:], in_=aps["degf"][:])
            nc.scalar.activation(dinv[:], dinv[:], AF.Sqrt)
            nc.vector.reciprocal(out=dinv[:], in_=dinv[:])
            emit_table_scale(nc, tc, pool, aps["l1full"], dinv, table2)
            aggT = pool.tile([HID, SHN], F32, tag="aggT", name="aggT_t")
            emit_geo(nc, tc, gmeta, aps, table2, aggT, iota)
            wt1 = pool.tile([HID, HID], F32, tag="wt", name="wt1_t")
            nc.sync.dma_start(out=wt1[:], in_=aps["wt1"][:])
            dinv_sh = pool.tile([P, SHN // P], F32, tag="dsh", name="dsh_t")
            nc.sync.dma_start(out=dinv_sh[:], in_=aps["degsh"][:])
            nc.scalar.activation(dinv_sh[:], dinv_sh[:], AF.Sqrt)
            nc.vector.reciprocal(out=dinv_sh[:], in_=dinv_sh[:])
            l2sh = pool.tile([P, SHN // P, HID], F32, tag="l2", name="l2_t")
            emit_shard_mm(nc, tc, pool, aggT, wt1, dinv_sh, l2sh)
            for b0 in range(0, SHN // P, 16):
                nb = min(16, SHN // P - b0)
                t1 = pool.tile([P, 16, HID], F32, tag="o1", name=f"o1{b0}")
                pv = aps["poish"].rearrange("(b p) d -> p b d", p=P)
                nc.sync.dma_start(out=t1[:, :nb, :], in_=pv[:, b0:b0 + nb, :])
                t2 = pool.tile([P, 16, HID], F32, tag="o2", name=f"o2{b0}")
                lv = aps["l1shr"].rearrange("(b p) d -> p b d", p=P)
                nc.sync.dma_start(out=t2[:, :nb, :], in_=lv[:, b0:b0 + nb, :])
                nc.vector.tensor_add(out=t1[:, :nb, :], in0=t1[:, :nb, :], in1=t2[:, :nb, :])
                nc.vector.tensor_add(out=t1[:, :nb, :], in0=t1[:, :nb, :],
                                     in1=l2sh[:, b0:b0 + nb, :])
                nc.scalar.mul(out=t1[:, :nb, :], in_=t1[:, :nb, :], mul=1.0 / 3.0)
                ov = geo_o.rearrange("(b p) d -> p b d", p=P)
                nc.sync.dma_start(out=ov[:, b0:b0 + nb, :], in_=t1[:, :nb, :])
    nc.compile()
    return nc


def kernel(poi_emb, distance_emb, temporal_emb, alpha_src, alpha_dst,
           W_geo, b_geo, geo_dist, geo_edge_index, sess_idx,
           seq_edge_index, edge_time, edge_dist):
    poi_emb = np.asarray(poi_emb, np.float32)
    geo_edge_index = np.asarray(geo_edge_index, np.int64)
    geo_dist = np.asarray(geo_dist, np.float32)
    sess_idx = np.asarray(sess_idx, np.int64)
    seq_edge_index = np.asarray(seq_edge_index, np.int64)
    edge_time = np.asarray(edge_time, np.int64)
    edge_dist_a = np.asarray(edge_dist, np.int64)

    poi_pad = np.zeros((NPAD, HID), np.float32)
    poi_pad[:NPOI] = poi_emb
    deg = np.bincount(geo_edge_index[1], minlength=NPOI) + 1
    deg_pad = np.ones(NPAD, np.float32)
    deg_pad[:NPOI] = deg
    degf = _wrap_data(deg_pad.reshape(NPAD // P, P).T.reshape(-1), np.float32)
    # careful: degf layout must match table row wrap (row = 128*b + p)
    degf = np.ascontiguousarray(deg_pad.reshape(NPAD // P, P).T).astype(np.float32)

    # ---- geo streams ----
    row = np.concatenate([geo_edge_index[0], np.arange(NPOI)])
    col = np.concatenate([geo_edge_index[1], np.arange(NPOI)])
    dist = np.concatenate([geo_dist, np.zeros(NPOI, np.float32)])
    shard = col // SHN
    kg, kw, arr_r, arr_d, arr_c = [], [], {"rowrel": [], "dist": [], "colrel": []}, None, None
    for c in range(N_CORES):
        m = shard == c
        r, cl, dd = row[m], col[m], dist[m]
        q = r // QN
        cll = cl - c * SHN
        w = cll // WIN
        kg.append(q)
        kw.append(w)
        arr_r["rowrel"].append((r - q * QN).astype(np.int64))
        arr_r["dist"].append(dd.astype(np.float32))
        arr_r["colrel"].append((cll - w * WIN).astype(np.float32))
    gseed = [g * 100000 + w for g in range(4) for w in range(SHN // WIN)]
    gout, g_tgrp, g_twin, g_calls, GL = _build_stream(
        kg, kw, arr_r, {"colrel": -1.0}, gseed)
    gmeta = {"L": GL, "calls": g_calls, "tile_grp": g_tgrp, "tile_win": g_twin}

    # ---- seq streams ----
    src, dst = seq_edge_index[0], seq_edge_index[1]
    j_all = np.concatenate([src, dst])
    i_all = np.concatenate([dst, src])
    d_all = np.concatenate([np.zeros(len(src), np.int64), np.ones(len(dst), np.int64)])
    e_all = np.concatenate([np.arange(len(src)), np.arange(len(src))])
    ish = i_all // SESS_SH
    ish[ish > 7] = 7
    skg, skw = [], []
    sarr = {"poijrel": [], "xrel": [], "edist": [], "etime": [], "colrel": []}
    xs_idx_list, sig_list = [], []
    for c in range(N_CORES):
        sess_lo = c * SESS_SH
        sl = sess_idx[sess_lo:sess_lo + SESS_SH]
        sig = np.argsort(sl // QN, kind="stable")
        inv = np.empty(SESS_SH, np.int64)
        inv[sig] = np.arange(SESS_SH)
        sig_list.append(sig)
        xs = np.zeros(SESS_PAD, np.int64)
        xs[:SESS_SH] = sl[sig] - (sl[sig] // QN) * QN
        xs_idx_list.append(xs)
        # x gather calls grouped by quarter (same structure all cores needed!)
        m = ish == c
        jm, im, dm, em = j_all[m], i_all[m], d_all[m], e_all[m]
        pj = sess_idx[jm]
        qj = pj // QN
        spos = inv[im - sess_lo]
        w = spos // WIN
        grp = dm * 4 + qj
        skg.append(grp)
        skw.append(w)
        sarr["poijrel"].append((pj - qj * QN).astype(np.int64))
        sarr["xrel"].append(spos.astype(np.int64))
        sarr["edist"].append(edge_dist_a[em].astype(np.int64))
        sarr["etime"].append(edge_time[em].astype(np.int64))
        sarr["colrel"].append((spos - w * WIN).astype(np.float32))
    sseed = [g * 100000 + w for g in range(8) for w in range(SESS_PAD // WIN)]
    sout, s_tgrp, s_twin, s_calls, SL = _build_stream(
        skg, skw, sarr, {"colrel": -1.0}, sseed)
    # x-shard gather calls: uniform via per-quarter max counts
    xqc = np.zeros((N_CORES, 4), np.int64)
    for c in range(N_CORES):
        q = (sess_idx[c * SESS_SH:(c + 1) * SESS_SH] // QN)
        for g in range(4):
            xqc[c, g] = (q == g).sum()
    xmax = [int(-(-xqc[:, g].max() // 128) * 128) for g in range(4)]
    XL = sum(xmax)
    xcalls = []
    pos = 0
    for g in range(4):
        off = 0
        while off < xmax[g]:
            n = min(NIDX, xmax[g] - off)
            xcalls.append((g, pos + off, n))
            off += n
        pos += xmax[g]
    smeta = {"L": SL, "calls": s_calls, "tile_grp": s_tgrp, "tile_win": s_twin,
             "xcalls": xcalls, "XL": XL}
    # per-core x_sidx padded to quarter-block structure
    xs_wrapped = []
    xpos_list = []
    for c in range(N_CORES):
        sl = sess_idx[c * SESS_SH:(c + 1) * SESS_SH]
        sig = sig_list[c]
        q_sorted = (sl[sig] // QN)
        buf = np.zeros(XL, np.int64)
        xpos = np.zeros(SESS_SH, np.int64)  # sigma-pos -> staged slot
        base = 0
        ptr = 0
        for g in range(4):
            cnt = int(xqc[c, g])
            buf[base:base + cnt] = (sl[sig] - q_sorted * QN)[ptr:ptr + cnt]
            xpos[ptr:ptr + cnt] = base + np.arange(cnt)
            ptr += cnt
            base += xmax[g]
        xs_wrapped.append(_wrap_idx(buf))
        xpos_list.append(xpos)
    # xrel must point at the STAGED slot (wrap layout [u%128, u//128])
    # staged row u lands at xsh row (u%128)*? no: we DMA tiles [128,k,64] to
    # xsh rows in (b p) order: row = 128*b + p = start + (u//128)*128?? tile
    # write xv[:, start/P + j, :] = gt[:, j, :] -> xsh row 128*(start/P+j)+p;
    # gathered msg u sits at [u%128, u//128] -> row = start + (u//128)*128 +
    # (u%128)?? NO: row index in (b p) wrap = 128*b + p with b = start//128 +
    # u//128, p = u%128 -> slot = start + 128*(u//128) + (u%128)... start is
    # mult of 128 so slot = start + u rearranged: slot_of_u = start +
    # (u//128)*128 + u%128 = start + u. Identity! Good: staged slot == buf pos.
    for c in range(N_CORES):
        spos = None  # xrel already = sigma position; remap to staged slot:
    # remap xrel arrays per core: staged_slot = xpos[sigma_pos]
    # (sout["xrel"] currently holds sigma positions)
    s_xrel_fixed = []
    for c in range(N_CORES):
        xr = sout["xrel"][c].astype(np.int64)
        xp = xpos_list[c]
        fixed = np.where(xr < SESS_SH, xp[np.clip(xr, 0, SESS_SH - 1)], 0)
        s_xrel_fixed.append(fixed)
    # NOTE: reduction windows are over sigma-pos space; output rows are in
    # sigma space?? No: colrel/w built from sigma positions (spos) -> seq
    # output row = sigma position. Host unpermutes via sig_list.

    iota256 = np.tile(np.arange(WIN, dtype=np.float32), (P, 1))
    alphas = np.tile(np.concatenate([np.asarray(alpha_src, np.float32),
                                     np.asarray(alpha_dst, np.float32)]), (P, 1))
    wt0 = np.ascontiguousarray(np.asarray(W_geo[0], np.float32).T)
    wt1 = np.ascontiguousarray(np.asarray(W_geo[1], np.float32).T)

    ncA = build_A(gmeta, smeta)
    in_maps = []
    for c in range(N_CORES):
        degsh = np.ascontiguousarray(
            deg_pad[c * SHN:(c + 1) * SHN].reshape(SHN // P, P).T).astype(np.float32)
        in_maps.append({
            "poi": poi_pad, "degf": degf, "degsh": degsh, "iota256": iota256,
            "g_idx": _wrap_idx(gout["rowrel"][c]),
            "g_dist": _wrap_data(gout["dist"][c], np.float32),
            "g_colrel": _wrap_data(gout["colrel"][c], np.float32),
            "wt0": wt0,
            "s_poij": _wrap_idx(sout["poijrel"][c]),
            "s_xrel": _wrap_idx(s_xrel_fixed[c]),
            "s_dist": _wrap_idx(sout["edist"][c]),
            "s_time": _wrap_idx(sout["etime"][c]),
            "s_colrel": _wrap_data(sout["colrel"][c], np.float32),
            "x_sidx": xs_wrapped[c],
            "alphas": alphas,
            "demb": np.asarray(distance_emb, np.float32),
            "temb": np.asarray(temporal_emb, np.float32),
        })
    import sys as _sys, time as _t
    _t0 = _t.perf_counter()
    resA = run_bass_kernel_spmd(ncA, in_maps, list(range(N_CORES)))
    print(f"[kernel] launch A run wall: {(_t.perf_counter()-_t0)*1e3:.1f} ms", file=_sys.stderr)

    l1_full = np.zeros((NPAD, HID), np.float32)
    for c in range(N_CORES):
        sh = resA.results[c]["l1sh"]
        l1_full[c * SHN:(c + 1) * SHN] = sh
    seq_out = np.zeros((NSESS, HID), np.float32)
    for c in range(N_CORES):
        sT = resA.results[c]["seqT"][:HID].T  # [SESS_PAD, 64] in sigma space
        sig = sig_list[c]
        out_rows = np.zeros((SESS_SH, HID), np.float32)
        out_rows[sig] = sT[:SESS_SH]
        seq_out[c * SESS_SH:(c + 1) * SESS_SH] = out_rows

    ncB = build_B(gmeta)
    in_mapsB = []
    for c in range(N_CORES):
        degsh = np.ascontiguousarray(
            deg_pad[c * SHN:(c + 1) * SHN].reshape(SHN // P, P).T).astype(np.float32)
        in_mapsB.append({
            "l1full": l1_full, "degf": degf, "degsh": degsh, "iota256": iota256,
            "g_idx": _wrap_idx(gout["rowrel"][c]),
            "g_dist": _wrap_data(gout["dist"][c], np.float32),
            "g_colrel": _wrap_data(gout["colrel"][c], np.float32),
            "wt1": wt1,
            "poish": poi_pad[c * SHN:(c + 1) * SHN],
            "l1shr": l1_full[c * SHN:(c + 1) * SHN],
        })
    _t0 = _t.perf_counter()
    resB = run_bass_kernel_spmd(ncB, in_mapsB, list(range(N_CORES)))
    print(f"[kernel] launch B run wall: {(_t.perf_counter()-_t0)*1e3:.1f} ms", file=_sys.stderr)
    geo_out = np.zeros((NPOI, HID), np.float32)
    for c in range(N_CORES):
        lo = c * SHN
        hi = min((c + 1) * SHN, NPOI)
        if lo < NPOI:
            geo_out[lo:hi] = resB.results[c]["geosh"][:hi - lo]
    return seq_out, geo_out
